# revision 34
# baseline (speedup 1.0000x reference)
"""Trainium2 Bass kernel for nn_Downsampler_47966194762291.

Data-parallel over batch: each of the 8 NeuronCores processes one image.

Math (derived from the reference, validated in numpy):
  With u[j] = j+0.5 broadcasting along the w axis, the gather coords are
  x0 = j+tx(k)+2, y0 = j+ty(k)+2 exactly (offsets in [0,1) -> no clamping,
  scl = 1), so the gathered pixels V[c,j,k] = img[c, j+tx+2, j+ty+2] are just
  5 diagonals of the image, independent of the output row i.
  The m1/m3 reshape pairs flat positions (2n, 2n+1): output rows i<128 use
  (1-f) and rows i>=128 use f at the same source positions.  Per target
  (i, j, k) the pair members live at fixed (plane, column-parity) slots of
  the offset tensors; the host pre-packs them (ae/ao/be/bo) in a k-major
  [p; r, k, jj] layout so every device op is a contiguous fp16 2x-mode op.

  Per half (lo=i<128 with complements, hi=i>=128 raw):
    S1 = sum_k G0*W0, S2 = sum_k G1*X, S3 = sum_k G0*Y, S4 = sum_k G1*W3
    out0 = S1+S2, out1 = S3+S2, out2 = S3+S4; softround at the end.
  where W0 = (a0+a1)V0, Y = a0V0+a1V1, X = a0V1+a1V2, W3 = (a0+a1)V2,
  G0 = K*b0, G1 = K*b1 (V pre-scaled by 255).  The 9-wide k-sum runs as
  identity-stationary accumulate-matmuls on the idle Tensor engine for
  the first column-half (hidden under the second half's DVE work) and as
  a 2x-mode pairwise fp16 TT-add tree on DVE for the second (its tail is
  exposed, and the DVE tree beats PE+PSUM-copy end-to-end there).

The reference's fp32 add-chain (oh+1.5+tx+u) rounds across the floor
boundary for a handful of offsets ~1.0 (tens of points per batch).  The
dense device path uses the raw offsets as bilinear fractions (error
<=1.6e-5 elsewhere); the affected output pixels are recomputed exactly on
the host by host-side fixup code below (input-dependent, not hardcoded).
"""
import math
import sys

sys.path.insert(0, "/opt/trn_rl_repo")

import numpy as np

import concourse.bacc as bacc
import concourse.bass as bass
import concourse.mybir as mybir
from concourse.tile import TileContext
from concourse.bass_utils import run_bass_kernel_spmd

F32 = mybir.dt.float32
F16 = mybir.dt.float16
AF = mybir.ActivationFunctionType
ALU = mybir.AluOpType

N_CORES = 8
PI2 = float(2.0 * math.pi)
MAGIC = 12582912.0  # 1.5 * 2^23: fp32 round-to-int magic

# pair mapping: for target k, even member (a0/b0) at (plane, 2jj+delta),
# odd member (a1/b1) likewise (see module docstring).
SRC0_PLANE = [0, 2, 4, 6, 8, 1, 3, 5, 7]
SRC0_DELTA = [0, 0, 0, 0, 0, 1, 1, 1, 1]
SRC1_PLANE = [1, 3, 5, 7, 0, 2, 4, 6, 8]
SRC1_DELTA = [0, 0, 0, 0, 1, 1, 1, 1, 1]

TAPS_X = np.repeat(np.arange(3), 3)
TAPS_Y = np.tile(np.arange(3), 3)


# ----------------------------------------------------------------------------
# device program
# ----------------------------------------------------------------------------

def build_program():
    nc = bacc.Bacc("TRN2", target_bir_lowering=False, debug=False,
                   num_devices=N_CORES)
    # [p](r, m(ae,ao,be,bo), k, jj) pre-packed pair weights, fp16
    abuf = nc.dram_tensor("abuf", [128, 9216], F16, kind="ExternalInput")
    # [p](half, r, k, jj) kernels, fp16
    kbuf = nc.dram_tensor("kbuf", [128, 4608], F16, kind="ExternalInput")
    # [128-replicated](t(V0,V1,V2,C12,C01), r, k, jj) 255-scaled, fp16
    vbuf = nc.dram_tensor("vbuf", [128, 11520], F16, kind="ExternalInput")
    # identity for PE accumulate-copy k-reduction
    ibuf = nc.dram_tensor("ibuf", [128, 128], F16, kind="ExternalInput")
    obuf = nc.dram_tensor("obuf", [3, 256, 256], F32, kind="ExternalOutput")

    def pap(t, off, stride, n, width):
        """[[pitch,128],[stride,n],[1,width]] view at element offset off."""
        return bass.AP(t.tensor, t.offset + off,
                       [[t.ap[0][0], 128], [stride, n], [1, width]])

    with TileContext(nc) as tc:
        with (
            tc.tile_pool(name="persist", bufs=1) as pp,
            tc.tile_pool(name="work", bufs=2) as wp,
            tc.tile_pool(name="early", bufs=1) as ep,
            tc.tile_pool(name="tail", bufs=1) as tp,
            tc.tile_pool(name="outp", bufs=2) as op_pool,
            tc.tile_pool(name="psum", bufs=2, space="PSUM") as psp,
        ):
            AB = pp.tile([128, 9216], F16, tag="AB")
            K = pp.tile([128, 4608], F16, tag="K")
            V = pp.tile([128, 11520], F16, tag="V")
            ID = pp.tile([128, 128], F16, tag="ID")

            # ---------------- loads (one HWDGE ring, priority order) -------
            abv = abuf.ap()
            vbv = vbuf.ap().rearrange("p (t r j) -> p t r j", t=5, r=2, j=1152)
            kbv = kbuf.ap().rearrange("p (h r j) -> p h r j", h=2, r=2, j=1152)
            Vv = V[:].rearrange("p (t r j) -> p t r j", t=5, r=2, j=1152)
            Kv = K[:].rearrange("p (h r j) -> p h r j", h=2, r=2, j=1152)

            # Single load ring (scalar HWDGE), strict priority order matched
            # to the jh0 op order (sE -> E -> G -> ...); stores on sync.
            nc.scalar.dma_start(out=AB[:, 0:2304], in_=abv[:, 0:2304])
            nc.scalar.dma_start(out=Vv[:, 0:3, 0], in_=vbv[:, 0:3, 0])
            nc.scalar.dma_start(out=AB[:, 2304:4608], in_=abv[:, 2304:4608])
            nc.scalar.dma_start(out=Kv[:, :, 0], in_=kbv[:, :, 0])
            nc.scalar.dma_start(out=Vv[:, 3:5, 0], in_=vbv[:, 3:5, 0])
            nc.scalar.dma_start(out=ID[:], in_=ibuf.ap())
            # r1: be|bo first so ACT's bb gets a head start
            nc.scalar.dma_start(out=AB[:, 6912:9216], in_=abv[:, 6912:9216])
            nc.scalar.dma_start(out=Vv[:, 0:3, 1], in_=vbv[:, 0:3, 1])
            nc.scalar.dma_start(out=AB[:, 4608:6912], in_=abv[:, 4608:6912])
            nc.scalar.dma_start(out=Kv[:, :, 1], in_=kbv[:, :, 1])
            nc.scalar.dma_start(out=Vv[:, 3:5, 1], in_=vbv[:, 3:5, 1])

            TT = nc.vector.tensor_tensor
            for jh in range(2):
                o = jh * 4608    # AB r-slice base
                vr = jh * 1152   # V r-slice base (within each table block)
                ko = jh * 1152   # K r-slice base (within half blocks)

                sE = ep.tile([128, 1152], F16, tag="sE", name="sE")
                nc.vector.tensor_add(sE[:], AB[:, o:o + 1152],
                                     AB[:, o + 1152:o + 2304])
                bb = wp.tile([128, 2304], F16, tag="bb", name="bb")
                nc.scalar.activation(bb[:], AB[:, o + 2304:o + 4608],
                                     AF.Copy, bias=1.0, scale=-1.0)

                # E = [ae*V0 ae*V1 | ao*V1 ao*V2] in one op: in0 repeats
                # ae,ae,ao,ao; in1 walks V0,V1 / V1,V2 (regular 2x2 grid)
                E = ep.tile([128, 4608], F16, tag="E", name="E")
                TT(E[:].rearrange("p (a b j) -> p a b j", a=2, b=2),
                   bass.AP(AB.tensor, AB.offset + o,
                           [[AB.ap[0][0], 128], [1152, 2], [0, 2], [1, 1152]]),
                   bass.AP(V.tensor, V.offset + vr,
                           [[V.ap[0][0], 128], [2304, 2], [2304, 2], [1, 1152]]),
                   op=ALU.mult)

                # G = [G0l G0h G1l G1h]
                G = wp.tile([128, 4608], F16, tag="G", name="G")
                TT(pap(G, 1152, 2304, 2, 1152), pap(K, 2304 + ko, 0, 2, 1152),
                   AB[:, o + 2304:o + 4608], op=ALU.mult)
                TT(pap(G, 0, 2304, 2, 1152), pap(K, ko, 0, 2, 1152),
                   bb[:], op=ALU.mult)

                # WYX = [W0l W0h | Xl Xh | Yl Yh | W3l W3h], 1152 each
                WYX = wp.tile([128, 9216], F16, tag="WYX", name="WYX")
                # Yh|Xh = (aeV0|aeV1) + (aoV1|aoV2)
                TT(pap(WYX, 4608 + 1152, -2304, 2, 1152), E[:, 0:2304],
                   E[:, 2304:4608], op=ALU.add)
                # W0h|W3h = sE*(V0|V2)
                TT(pap(WYX, 1152, 6912, 2, 1152), pap(sE, 0, 0, 2, 1152),
                   pap(V, vr, 4608, 2, 1152), op=ALU.mult)
                # Xl|Yl = (C12|C01) - (Xh|Yh)
                TT(pap(WYX, 2304, 2304, 2, 1152),
                   pap(V, 3 * 2304 + vr, 2304, 2, 1152),
                   pap(WYX, 2304 + 1152, 2304, 2, 1152), op=ALU.subtract)
                # W0l|W3l = (2-sE)*(V0|V2); sEl runs in 4x single-src mode
                sEl = ep.tile([128, 1152], F16, tag="sEl", name="sEl")
                nc.vector.tensor_scalar(sEl[:], sE[:], -1.0, 2.0,
                                        ALU.mult, ALU.add)
                TT(pap(WYX, 0, 6912, 2, 1152), pap(sEl, 0, 0, 2, 1152),
                   pap(V, vr, 4608, 2, 1152), op=ALU.mult)

                # U = [G*(W0l W0h Xl Xh) | G*(Yl Yh W3l W3h)] in one op:
                # in1 blocks (W0@0, X@2304 | Y@4608, W3@6912) form a 2x2 grid
                U = wp.tile([128, 9216], F16, tag="U", name="U")
                TT(U[:].rearrange("p (a b j) -> p a b j", a=2, b=2),
                   bass.AP(G.tensor, G.offset,
                           [[G.ap[0][0], 128], [0, 2], [2304, 2], [1, 2304]]),
                   bass.AP(WYX.tensor, WYX.offset,
                           [[WYX.ap[0][0], 128], [4608, 2], [2304, 2],
                            [1, 2304]]),
                   op=ALU.mult)

                # k-sum: R = [S1l S1h S2l S2h | S3l S3h S4l S4h]
                if jh == 0:
                    # Tensor engine: accumulate the 9 k-planes of 4 blocks at
                    # a time into PSUM via identity matmuls (hidden under
                    # jh1's DVE work), then stage to SBUF on ACT (DVE can
                    # read only one PSUM operand).
                    R = psp.tile([128, 1024], F32, tag="Rps", name="Rps")
                    for s in range(2):
                        for k in range(9):
                            nc.tensor.matmul(
                                R[:, s * 512:(s + 1) * 512], ID[:],
                                bass.AP(U.tensor,
                                        U.offset + s * 4608 + k * 128,
                                        [[U.ap[0][0], 128], [1152, 4],
                                         [1, 128]]),
                                start=(k == 0), stop=(k == 8))
                    Rs = tp.tile([128, 1024], F32, tag="Rsb", name="Rsb")
                    nc.scalar.activation(Rs[:], R[:], AF.Copy)
                else:
                    # jh1's reduction tail is fully exposed: the fp16
                    # pairwise DVE tree is faster end-to-end than PE+copy.
                    T1 = tp.tile([128, 4096], F16, tag="T1", name="T1")
                    TT(bass.AP(T1.tensor, T1.offset,
                               [[T1.ap[0][0], 128], [1024, 4], [128, 8],
                                [1, 128]]),
                       bass.AP(U.tensor, U.offset,
                               [[U.ap[0][0], 128], [256, 4], [1152, 8],
                                [1, 128]]),
                       bass.AP(U.tensor, U.offset + 128,
                               [[U.ap[0][0], 128], [256, 4], [1152, 8],
                                [1, 128]]),
                       op=ALU.add)
                    T2 = tp.tile([128, 2048], F16, tag="T2", name="T2")
                    TT(T2[:], pap(T1, 0, 2048, 2, 1024),
                       pap(T1, 1024, 2048, 2, 1024), op=ALU.add)
                    T3 = tp.tile([128, 1024], F16, tag="T3", name="T3")
                    TT(T3[:], T2[:, 0:1024], T2[:, 1024:2048], op=ALU.add)
                    Rs = tp.tile([128, 1024], F16, tag="R16", name="R16")
                    TT(pap(Rs, 0, 128, 8, 128), pap(T3, 0, 128, 8, 128),
                       pap(U, 8 * 128, 1152, 8, 128), op=ALU.add)

                # combines: out0=S1+S2, out1=S3+S2, out2=S3+S4
                # layout [c, half, jj] so the store AP merges cleanly.
                outJ = op_pool.tile([128, 768], F32, tag="outJ", name="outJ")
                # out0 & out2 share one stride grid; out1 separate
                TT(bass.AP(outJ.tensor, outJ.offset,
                           [[outJ.ap[0][0], 128], [512, 2], [128, 2],
                            [1, 128]]),
                   bass.AP(Rs.tensor, Rs.offset,
                           [[Rs.ap[0][0], 128], [512, 2], [128, 2], [1, 128]]),
                   bass.AP(Rs.tensor, Rs.offset + 256,
                           [[Rs.ap[0][0], 128], [512, 2], [128, 2], [1, 128]]),
                   op=ALU.add)
                TT(pap(outJ, 256, 128, 2, 128), pap(Rs, 512, 128, 2, 128),
                   pap(Rs, 256, 128, 2, 128), op=ALU.add)

                # softround: out -= sin(2*pi*out)/(2*pi), Sin needs [-pi,pi]
                frt = tp.tile([128, 768], F32, tag="frt", name="frt")
                nc.vector.tensor_scalar(frt[:], outJ[:], MAGIC, MAGIC,
                                        ALU.add, ALU.subtract)
                nc.vector.tensor_sub(frt[:], outJ[:], frt[:])
                sin_t = tp.tile([128, 768], F32, tag="sin", name="sin")
                nc.scalar.activation(sin_t[:], frt[:], AF.Sin, scale=-PI2)
                nc.vector.scalar_tensor_tensor(outJ[:], sin_t[:], 1.0 / PI2,
                                               outJ[:], ALU.mult, ALU.add)

                # store: [p; c, half, jj] -> obuf[c, 128*half+p, 128*jh+jj]
                dst = bass.AP(obuf, jh * 128,
                              [[256, 128], [65536, 3], [32768, 2], [1, 128]])
                nc.sync.dma_start(
                    out=dst,
                    in_=outJ[:].rearrange("p (c h j) -> p c h j", c=3, h=2))

    nc.compile()
    return nc


_cached_nc = None


def _get_nc():
    global _cached_nc
    if _cached_nc is None:
        _cached_nc = build_program()
    return _cached_nc


# ----------------------------------------------------------------------------
# host-side exact fixup for floor-boundary crossings (sparse, input-dependent)
# ----------------------------------------------------------------------------

SCALE, KS = 2, 3
K2 = KS * KS


def _chain(off_t, taps, u):
    t1 = (off_t + np.float32(KS / 2)).astype(np.float32)
    t2 = (t1 + taps).astype(np.float32)
    return (t2 + u[None, None, :, None]).astype(np.float32)


def _cx_at(off_t, taps, u, b, ii, jj, kk):
    v = off_t[b, ii, jj, kk]
    t1 = (v + np.float32(KS / 2)).astype(np.float32)
    t2 = (t1 + taps[kk]).astype(np.float32)
    return (t2 + u[jj]).astype(np.float32)


def _apply_fixup(out, img, kernels, offsets_h, offsets_v):
    B, C, H, W = img.shape
    h, w = H // SCALE, W // SCALE
    N = h * w * K2
    u = (np.arange(h, dtype=np.float32) + np.float32(0.5 * SCALE - 0.5))
    oh_t = offsets_h.transpose(0, 2, 3, 1)
    ov_t = offsets_v.transpose(0, 2, 3, 1)
    tx = TAPS_X.astype(np.float32)
    ty = TAPS_Y.astype(np.float32)
    jgrid = np.arange(w)[None, None, :, None]
    ex = np.floor(_chain(oh_t, tx, u)).astype(np.int64) != (
        jgrid + TAPS_X.astype(np.int64) + 2)
    ey = np.floor(_chain(ov_t, ty, u)).astype(np.int64) != (
        jgrid + TAPS_Y.astype(np.int64) + 2)
    pts = np.argwhere(ex | ey)
    if len(pts) == 0:
        return out
    affected = set()
    for b, i, j, k in pts:
        affected.add((b, i, j))
        n = (i * w + j) * K2 + k
        p = n // 2
        affected.add((b, p // (K2 * w), (p // K2) % w))
        affected.add((b, p // (K2 * w) + h // 2, (p // K2) % w))
    half = N // 2
    for b, i, j in sorted(affected):
        acc = np.zeros(3, np.float64)
        for k in range(K2):
            n = (i * w + j) * K2 + k
            if n < half:
                m0, m1, comp = 2 * n, 2 * n + 1, True
            else:
                m0, m1, comp = 2 * n - N, 2 * n - N + 1, False

            def coeff(m, off_t, taps):
                ii = m // (K2 * w); jj = (m // K2) % w; kk = m % K2
                t3 = _cx_at(off_t, taps, u, b, ii, jj, kk)
                fr = np.float32(t3 - np.floor(t3))
                return np.float32(1.0) - fr if comp else fr

            a0 = coeff(m0, oh_t, tx); a1 = coeff(m1, oh_t, tx)
            b0 = coeff(m0, ov_t, ty); b1 = coeff(m1, ov_t, ty)
            x0 = np.clip(int(np.floor(_cx_at(oh_t, tx, u, b, i, j, k))), 0, W - 1)
            y0 = np.clip(int(np.floor(_cx_at(ov_t, ty, u, b, i, j, k))), 0, H - 1)
            V0, V1, V2 = img[b, 0, x0, y0], img[b, 1, x0, y0], img[b, 2, x0, y0]
            res0 = b0 * (a0 * V0 + a1 * V0) + b1 * (a0 * V1 + a1 * V2)
            res1 = b0 * (a0 * V0 + a1 * V1) + b1 * (a0 * V1 + a1 * V2)
            res2 = b0 * (a0 * V0 + a1 * V1) + b1 * (a0 * V2 + a1 * V2)
            acc += kernels[b, k, i, j] * np.array([res0, res1, res2])
        o = np.float32(acc * 255.0)
        out[b, i, j, :] = o - np.sin(np.float32(2 * np.pi) * o) / np.float32(2 * np.pi)
    return out


# ----------------------------------------------------------------------------
# host-side input packing
# ----------------------------------------------------------------------------

def _pack_inputs(img, kernels, offsets_h, offsets_v):
    B = img.shape[0]
    # A: (B, p, r, m(ae,ao,be,bo), k, jj) fp16
    A = np.empty((B, 128, 2, 4, 9, 128), np.float16)
    ohv = offsets_h.reshape(B, 9, 128, 2, 128, 2)  # (b, plane, p, r, jj, t)
    ovv = offsets_v.reshape(B, 9, 128, 2, 128, 2)
    for k in range(9):
        A[:, :, :, 0, k, :] = ohv[:, SRC0_PLANE[k], :, :, :, SRC0_DELTA[k]]
        A[:, :, :, 1, k, :] = ohv[:, SRC1_PLANE[k], :, :, :, SRC1_DELTA[k]]
        A[:, :, :, 2, k, :] = ovv[:, SRC0_PLANE[k], :, :, :, SRC0_DELTA[k]]
        A[:, :, :, 3, k, :] = ovv[:, SRC1_PLANE[k], :, :, :, SRC1_DELTA[k]]
    A = A.reshape(B, 128, 9216)

    # K: (B, p, half, r, k, jj) fp16
    Kp = kernels.reshape(B, 9, 2, 128, 2, 128).transpose(0, 3, 2, 4, 1, 5)
    Kp = np.ascontiguousarray(Kp.astype(np.float16)).reshape(B, 128, 4608)

    # V tables: (B, t(V0,V1,V2,C01,C12,2V0,2V2), r, k, jj) * 255, fp16,
    # replicated to 32 partitions (device doubles 32->64->128)
    Vt = np.empty((B, 3, 9, 256), np.float32)
    j = np.arange(256)
    for k in range(9):
        Vt[:, :, k, :] = img[:, :, j + TAPS_X[k] + 2, j + TAPS_Y[k] + 2]
    Vt *= 255.0
    # table order: V0 V1 V2 | C12 C01 (C pair is the lo-subtract's in0)
    V5 = np.empty((B, 5, 9, 256), np.float32)
    V5[:, 0:3] = Vt
    V5[:, 3] = Vt[:, 1] + Vt[:, 2]
    V5[:, 4] = Vt[:, 0] + Vt[:, 1]
    # (b, t, k, j) -> (b, t, r, k, jj)
    V5 = V5.reshape(B, 5, 9, 2, 128).transpose(0, 1, 3, 2, 4)
    V5 = V5.astype(np.float16).reshape(B, 1, 11520)
    Vrep = np.ascontiguousarray(np.broadcast_to(V5, (B, 128, 11520)))
    return A, Kp, Vrep


# ----------------------------------------------------------------------------
# entry point
# ----------------------------------------------------------------------------

def kernel(img, kernels, offsets_h, offsets_v):
    img = np.ascontiguousarray(img, np.float32)
    kernels = np.ascontiguousarray(kernels, np.float32)
    offsets_h = np.ascontiguousarray(offsets_h, np.float32)
    offsets_v = np.ascontiguousarray(offsets_v, np.float32)

    A, Kp, Vrep = _pack_inputs(img, kernels, offsets_h, offsets_v)

    nc = _get_nc()
    ident = np.ascontiguousarray(np.eye(128, dtype=np.float16))
    in_maps = [
        {
            "abuf": np.ascontiguousarray(A[b]),
            "kbuf": np.ascontiguousarray(Kp[b]),
            "vbuf": Vrep[b],
            "ibuf": ident,
        }
        for b in range(N_CORES)
    ]
    res = run_bass_kernel_spmd(nc, in_maps, list(range(N_CORES)))
    out = np.stack([res.results[b]["obuf"] for b in range(N_CORES)])  # (8,3,h,w)
    out = np.ascontiguousarray(out.transpose(0, 2, 3, 1))             # (8,h,w,3)
    out = _apply_fixup(out, img, kernels, offsets_h, offsets_v)
    return out.astype(np.float32)


# revision 37
# speedup vs baseline: 1.0534x; 1.0534x over previous
"""Trainium2 Bass kernel for nn_Downsampler_47966194762291.

Data-parallel over batch: each of the 8 NeuronCores processes one image.

Math (derived from the reference, validated in numpy):
  With u[j] = j+0.5 broadcasting along the w axis, the gather coords are
  x0 = j+tx(k)+2, y0 = j+ty(k)+2 exactly (offsets in [0,1) -> no clamping,
  scl = 1), so the gathered pixels V[c,j,k] = img[c, j+tx+2, j+ty+2] are just
  5 diagonals of the image, independent of the output row i.
  The m1/m3 reshape pairs flat positions (2n, 2n+1): output rows i<128 use
  (1-f) and rows i>=128 use f at the same source positions.  Per target
  (i, j, k) the pair members live at fixed (plane, column-parity) slots of
  the offset tensors; the host pre-packs them (ae/ao/be/bo) in a k-major
  [p; r, k, jj] layout so every device op is a contiguous fp16 2x-mode op.

  Per half (lo=i<128 with complements, hi=i>=128 raw):
    S1 = sum_k G0*W0, S2 = sum_k G1*X, S3 = sum_k G0*Y, S4 = sum_k G1*W3
    out0 = S1+S2, out1 = S3+S2, out2 = S3+S4; softround at the end.
  where W0 = (a0+a1)V0, Y = a0V0+a1V1, X = a0V1+a1V2, W3 = (a0+a1)V2,
  G0 = K*b0, G1 = K*b1 (V pre-scaled by 255).  The 9-wide k-sum runs as
  identity-stationary accumulate-matmuls on the idle Tensor engine for
  the first column-half (hidden under the second half's DVE work) and as
  a 2x-mode pairwise fp16 TT-add tree on DVE for the second (its tail is
  exposed, and the DVE tree beats PE+PSUM-copy end-to-end there).

The reference's fp32 add-chain (oh+1.5+tx+u) rounds across the floor
boundary for a handful of offsets ~1.0 (tens of points per batch).  The
dense device path uses the raw offsets as bilinear fractions (error
<=1.6e-5 elsewhere); the affected output pixels are recomputed exactly on
the host by host-side fixup code below (input-dependent, not hardcoded).
"""
import math
import sys

sys.path.insert(0, "/opt/trn_rl_repo")

import numpy as np

import concourse.bacc as bacc
import concourse.bass as bass
import concourse.mybir as mybir
from concourse.tile import TileContext
from concourse.bass_utils import run_bass_kernel_spmd

F32 = mybir.dt.float32
F16 = mybir.dt.float16
AF = mybir.ActivationFunctionType
ALU = mybir.AluOpType

N_CORES = 8
PI2 = float(2.0 * math.pi)
MAGIC = 12582912.0  # 1.5 * 2^23: fp32 round-to-int magic

# pair mapping: for target k, even member (a0/b0) at (plane, 2jj+delta),
# odd member (a1/b1) likewise (see module docstring).
SRC0_PLANE = [0, 2, 4, 6, 8, 1, 3, 5, 7]
SRC0_DELTA = [0, 0, 0, 0, 0, 1, 1, 1, 1]
SRC1_PLANE = [1, 3, 5, 7, 0, 2, 4, 6, 8]
SRC1_DELTA = [0, 0, 0, 0, 1, 1, 1, 1, 1]

TAPS_X = np.repeat(np.arange(3), 3)
TAPS_Y = np.tile(np.arange(3), 3)


# ----------------------------------------------------------------------------
# device program
# ----------------------------------------------------------------------------

def build_program():
    nc = bacc.Bacc("TRN2", target_bir_lowering=False, debug=False,
                   num_devices=N_CORES)
    # [p](r, m(ae,ao,be,bo), k, jj) pre-packed pair weights, fp16
    abuf = nc.dram_tensor("abuf", [128, 9216], F16, kind="ExternalInput")
    # [p](half, r, k, jj) kernels, fp16
    kbuf = nc.dram_tensor("kbuf", [128, 4608], F16, kind="ExternalInput")
    # [128-replicated](t(V0,V1,V2,C12,C01), r, k, jj) 255-scaled, fp16
    vbuf = nc.dram_tensor("vbuf", [128, 11520], F16, kind="ExternalInput")
    # identity for PE accumulate-copy k-reduction
    ibuf = nc.dram_tensor("ibuf", [128, 128], F16, kind="ExternalInput")
    obuf = nc.dram_tensor("obuf", [3, 256, 256], F32, kind="ExternalOutput")

    def pap(t, off, stride, n, width):
        """[[pitch,128],[stride,n],[1,width]] view at element offset off."""
        return bass.AP(t.tensor, t.offset + off,
                       [[t.ap[0][0], 128], [stride, n], [1, width]])

    with TileContext(nc) as tc:
        with (
            tc.tile_pool(name="persist", bufs=1) as pp,
            tc.tile_pool(name="work", bufs=2) as wp,
            tc.tile_pool(name="early", bufs=1) as ep,
            tc.tile_pool(name="tail", bufs=1) as tp,
            tc.tile_pool(name="outp", bufs=2) as op_pool,
            tc.tile_pool(name="psum", bufs=2, space="PSUM") as psp,
        ):
            AB = pp.tile([128, 9216], F16, tag="AB")
            K = pp.tile([128, 4608], F16, tag="K")
            V = pp.tile([128, 11520], F16, tag="V")
            ID = pp.tile([128, 128], F16, tag="ID")

            # ---------------- loads (one HWDGE ring, priority order) -------
            abv = abuf.ap()
            vbv = vbuf.ap().rearrange("p (t r j) -> p t r j", t=5, r=2, j=1152)
            kbv = kbuf.ap().rearrange("p (h r j) -> p h r j", h=2, r=2, j=1152)
            Vv = V[:].rearrange("p (t r j) -> p t r j", t=5, r=2, j=1152)
            Kv = K[:].rearrange("p (h r j) -> p h r j", h=2, r=2, j=1152)

            # Single load ring (scalar HWDGE), strict priority order matched
            # to the jh0 op order (sE -> E -> G -> ...); stores on sync.
            nc.scalar.dma_start(out=AB[:, 0:2304], in_=abv[:, 0:2304])
            nc.scalar.dma_start(out=Vv[:, 0:3, 0], in_=vbv[:, 0:3, 0])
            nc.scalar.dma_start(out=AB[:, 2304:4608], in_=abv[:, 2304:4608])
            nc.scalar.dma_start(out=Kv[:, :, 0], in_=kbv[:, :, 0])
            nc.scalar.dma_start(out=Vv[:, 3:5, 0], in_=vbv[:, 3:5, 0])
            nc.scalar.dma_start(out=ID[:], in_=ibuf.ap())
            # r1: be|bo first so ACT's bb gets a head start
            nc.scalar.dma_start(out=AB[:, 6912:9216], in_=abv[:, 6912:9216])
            nc.scalar.dma_start(out=Vv[:, 0:3, 1], in_=vbv[:, 0:3, 1])
            nc.scalar.dma_start(out=AB[:, 4608:6912], in_=abv[:, 4608:6912])
            nc.scalar.dma_start(out=Kv[:, :, 1], in_=kbv[:, :, 1])
            nc.scalar.dma_start(out=Vv[:, 3:5, 1], in_=vbv[:, 3:5, 1])

            TT = nc.vector.tensor_tensor
            for jh in range(2):
                o = jh * 4608    # AB r-slice base
                vr = jh * 1152   # V r-slice base (within each table block)
                ko = jh * 1152   # K r-slice base (within half blocks)

                sE = ep.tile([128, 1152], F16, tag="sE", name="sE")
                nc.vector.tensor_add(sE[:], AB[:, o:o + 1152],
                                     AB[:, o + 1152:o + 2304])
                bb = wp.tile([128, 2304], F16, tag="bb", name="bb")
                nc.scalar.activation(bb[:], AB[:, o + 2304:o + 4608],
                                     AF.Copy, bias=1.0, scale=-1.0)

                # E = [ae*V0 ae*V1 | ao*V1 ao*V2] in one op: in0 repeats
                # ae,ae,ao,ao; in1 walks V0,V1 / V1,V2 (regular 2x2 grid)
                E = ep.tile([128, 4608], F16, tag="E", name="E")
                TT(E[:].rearrange("p (a b j) -> p a b j", a=2, b=2),
                   bass.AP(AB.tensor, AB.offset + o,
                           [[AB.ap[0][0], 128], [1152, 2], [0, 2], [1, 1152]]),
                   bass.AP(V.tensor, V.offset + vr,
                           [[V.ap[0][0], 128], [2304, 2], [2304, 2], [1, 1152]]),
                   op=ALU.mult)

                # G = [G0l G0h G1l G1h]
                G = wp.tile([128, 4608], F16, tag="G", name="G")
                TT(pap(G, 1152, 2304, 2, 1152), pap(K, 2304 + ko, 0, 2, 1152),
                   AB[:, o + 2304:o + 4608], op=ALU.mult)
                TT(pap(G, 0, 2304, 2, 1152), pap(K, ko, 0, 2, 1152),
                   bb[:], op=ALU.mult)

                # WYX = [W0l W0h | Xl Xh | Yl Yh | W3l W3h], 1152 each
                WYX = wp.tile([128, 9216], F16, tag="WYX", name="WYX")
                # Yh|Xh = (aeV0|aeV1) + (aoV1|aoV2)
                TT(pap(WYX, 4608 + 1152, -2304, 2, 1152), E[:, 0:2304],
                   E[:, 2304:4608], op=ALU.add)
                # W0h|W3h = sE*(V0|V2)
                TT(pap(WYX, 1152, 6912, 2, 1152), pap(sE, 0, 0, 2, 1152),
                   pap(V, vr, 4608, 2, 1152), op=ALU.mult)
                # Xl|Yl = (C12|C01) - (Xh|Yh)
                TT(pap(WYX, 2304, 2304, 2, 1152),
                   pap(V, 3 * 2304 + vr, 2304, 2, 1152),
                   pap(WYX, 2304 + 1152, 2304, 2, 1152), op=ALU.subtract)
                # W0l|W3l = (2-sE)*(V0|V2); sEl runs in 4x single-src mode
                sEl = ep.tile([128, 1152], F16, tag="sEl", name="sEl")
                nc.vector.tensor_scalar(sEl[:], sE[:], -1.0, 2.0,
                                        ALU.mult, ALU.add)
                TT(pap(WYX, 0, 6912, 2, 1152), pap(sEl, 0, 0, 2, 1152),
                   pap(V, vr, 4608, 2, 1152), op=ALU.mult)

                # U12 = G*(W0l W0h Xl Xh), U34 = G*(Yl Yh W3l W3h); both are
                # contiguous WYX slices.  jh0 runs them as one op; jh1 splits
                # so the PE can start reducing set0 while DVE builds set1.
                U = wp.tile([128, 9216], F16, tag="U", name="U")
                if jh == 0:
                    TT(U[:].rearrange("p (a b j) -> p a b j", a=2, b=2),
                       bass.AP(G.tensor, G.offset,
                               [[G.ap[0][0], 128], [0, 2], [2304, 2],
                                [1, 2304]]),
                       bass.AP(WYX.tensor, WYX.offset,
                               [[WYX.ap[0][0], 128], [4608, 2], [2304, 2],
                                [1, 2304]]),
                       op=ALU.mult)
                else:
                    TT(U[:, 0:4608], G[:], WYX[:, 0:4608], op=ALU.mult)
                    TT(U[:, 4608:9216], G[:], WYX[:, 4608:9216], op=ALU.mult)

                # k-sum: R = [S1l S1h S2l S2h | S3l S3h S4l S4h]
                if jh == 0:
                    # Tensor engine: accumulate the 9 k-planes of 4 blocks at
                    # a time into PSUM via identity matmuls (hidden under
                    # jh1's DVE work), then stage to SBUF on ACT (DVE can
                    # read only one PSUM operand).
                    R = psp.tile([128, 1024], F32, tag="Rps", name="Rps")
                    for s in range(2):
                        for k in range(9):
                            nc.tensor.matmul(
                                R[:, s * 512:(s + 1) * 512], ID[:],
                                bass.AP(U.tensor,
                                        U.offset + s * 4608 + k * 128,
                                        [[U.ap[0][0], 128], [1152, 4],
                                         [1, 128]]),
                                start=(k == 0), stop=(k == 8))
                    Rs = tp.tile([128, 1024], F32, tag="Rsb", name="Rsb")
                    nc.scalar.activation(Rs[:], R[:], AF.Copy)
                else:
                    # jh1's tail is exposed: reduce set0 (S1,S2) on the PE
                    # right after U12 (overlaps DVE's U34 + set1 tree), and
                    # set1 (S3,S4) via the fp16 pairwise DVE tree.
                    Rp1 = psp.tile([128, 512], F32, tag="Rps1", name="Rps1")
                    for k in range(9):
                        nc.tensor.matmul(
                            Rp1[:, 0:512], ID[:],
                            bass.AP(U.tensor, U.offset + k * 128,
                                    [[U.ap[0][0], 128], [1152, 4], [1, 128]]),
                            start=(k == 0), stop=(k == 8))
                    Rs = tp.tile([128, 512], F32, tag="Rsb1", name="Rsb1")
                    nc.scalar.activation(Rs[:], Rp1[:], AF.Copy)

                    T1 = tp.tile([128, 2048], F16, tag="T1", name="T1")
                    TT(bass.AP(T1.tensor, T1.offset,
                               [[T1.ap[0][0], 128], [512, 4], [128, 4],
                                [1, 128]]),
                       bass.AP(U.tensor, U.offset + 4608,
                               [[U.ap[0][0], 128], [256, 4], [1152, 4],
                                [1, 128]]),
                       bass.AP(U.tensor, U.offset + 4608 + 128,
                               [[U.ap[0][0], 128], [256, 4], [1152, 4],
                                [1, 128]]),
                       op=ALU.add)
                    T2 = tp.tile([128, 1024], F16, tag="T2", name="T2")
                    TT(T2[:], pap(T1, 0, 1024, 2, 512),
                       pap(T1, 512, 1024, 2, 512), op=ALU.add)
                    T3 = tp.tile([128, 512], F16, tag="T3", name="T3")
                    TT(T3[:], T2[:, 0:512], T2[:, 512:1024], op=ALU.add)
                    R16 = tp.tile([128, 512], F16, tag="R16", name="R16")
                    TT(pap(R16, 0, 128, 4, 128), pap(T3, 0, 128, 4, 128),
                       pap(U, 4608 + 8 * 128, 1152, 4, 128), op=ALU.add)

                # combines: out0=S1+S2, out1=S3+S2, out2=S3+S4
                # layout [c, half, jj] so the store AP merges cleanly.
                outJ = op_pool.tile([128, 768], F32, tag="outJ", name="outJ")
                if jh == 0:
                    # out0 & out2 share one stride grid; out1 separate
                    TT(bass.AP(outJ.tensor, outJ.offset,
                               [[outJ.ap[0][0], 128], [512, 2], [128, 2],
                                [1, 128]]),
                       bass.AP(Rs.tensor, Rs.offset,
                               [[Rs.ap[0][0], 128], [512, 2], [128, 2],
                                [1, 128]]),
                       bass.AP(Rs.tensor, Rs.offset + 256,
                               [[Rs.ap[0][0], 128], [512, 2], [128, 2],
                                [1, 128]]),
                       op=ALU.add)
                    TT(pap(outJ, 256, 128, 2, 128), pap(Rs, 512, 128, 2, 128),
                       pap(Rs, 256, 128, 2, 128), op=ALU.add)
                else:
                    TT(pap(outJ, 0, 128, 2, 128), pap(Rs, 0, 128, 2, 128),
                       pap(Rs, 256, 128, 2, 128), op=ALU.add)
                    TT(pap(outJ, 256, 128, 2, 128), pap(R16, 0, 128, 2, 128),
                       pap(Rs, 256, 128, 2, 128), op=ALU.add)
                    TT(pap(outJ, 512, 128, 2, 128), pap(R16, 0, 128, 2, 128),
                       pap(R16, 256, 128, 2, 128), op=ALU.add)

                # softround: out -= sin(2*pi*out)/(2*pi), Sin needs [-pi,pi]
                frt = tp.tile([128, 768], F32, tag="frt", name="frt")
                nc.vector.tensor_scalar(frt[:], outJ[:], MAGIC, MAGIC,
                                        ALU.add, ALU.subtract)
                nc.vector.tensor_sub(frt[:], outJ[:], frt[:])
                sin_t = tp.tile([128, 768], F32, tag="sin", name="sin")
                nc.scalar.activation(sin_t[:], frt[:], AF.Sin, scale=-PI2)
                nc.vector.scalar_tensor_tensor(outJ[:], sin_t[:], 1.0 / PI2,
                                               outJ[:], ALU.mult, ALU.add)

                # store: [p; c, half, jj] -> obuf[c, 128*half+p, 128*jh+jj]
                dst = bass.AP(obuf, jh * 128,
                              [[256, 128], [65536, 3], [32768, 2], [1, 128]])
                nc.sync.dma_start(
                    out=dst,
                    in_=outJ[:].rearrange("p (c h j) -> p c h j", c=3, h=2))

    nc.compile()
    return nc


_cached_nc = None


def _get_nc():
    global _cached_nc
    if _cached_nc is None:
        _cached_nc = build_program()
    return _cached_nc


# ----------------------------------------------------------------------------
# host-side exact fixup for floor-boundary crossings (sparse, input-dependent)
# ----------------------------------------------------------------------------

SCALE, KS = 2, 3
K2 = KS * KS


def _chain(off_t, taps, u):
    t1 = (off_t + np.float32(KS / 2)).astype(np.float32)
    t2 = (t1 + taps).astype(np.float32)
    return (t2 + u[None, None, :, None]).astype(np.float32)


def _cx_at(off_t, taps, u, b, ii, jj, kk):
    v = off_t[b, ii, jj, kk]
    t1 = (v + np.float32(KS / 2)).astype(np.float32)
    t2 = (t1 + taps[kk]).astype(np.float32)
    return (t2 + u[jj]).astype(np.float32)


def _apply_fixup(out, img, kernels, offsets_h, offsets_v):
    B, C, H, W = img.shape
    h, w = H // SCALE, W // SCALE
    N = h * w * K2
    u = (np.arange(h, dtype=np.float32) + np.float32(0.5 * SCALE - 0.5))
    oh_t = offsets_h.transpose(0, 2, 3, 1)
    ov_t = offsets_v.transpose(0, 2, 3, 1)
    tx = TAPS_X.astype(np.float32)
    ty = TAPS_Y.astype(np.float32)
    jgrid = np.arange(w)[None, None, :, None]
    ex = np.floor(_chain(oh_t, tx, u)).astype(np.int64) != (
        jgrid + TAPS_X.astype(np.int64) + 2)
    ey = np.floor(_chain(ov_t, ty, u)).astype(np.int64) != (
        jgrid + TAPS_Y.astype(np.int64) + 2)
    pts = np.argwhere(ex | ey)
    if len(pts) == 0:
        return out
    affected = set()
    for b, i, j, k in pts:
        affected.add((b, i, j))
        n = (i * w + j) * K2 + k
        p = n // 2
        affected.add((b, p // (K2 * w), (p // K2) % w))
        affected.add((b, p // (K2 * w) + h // 2, (p // K2) % w))
    half = N // 2
    for b, i, j in sorted(affected):
        acc = np.zeros(3, np.float64)
        for k in range(K2):
            n = (i * w + j) * K2 + k
            if n < half:
                m0, m1, comp = 2 * n, 2 * n + 1, True
            else:
                m0, m1, comp = 2 * n - N, 2 * n - N + 1, False

            def coeff(m, off_t, taps):
                ii = m // (K2 * w); jj = (m // K2) % w; kk = m % K2
                t3 = _cx_at(off_t, taps, u, b, ii, jj, kk)
                fr = np.float32(t3 - np.floor(t3))
                return np.float32(1.0) - fr if comp else fr

            a0 = coeff(m0, oh_t, tx); a1 = coeff(m1, oh_t, tx)
            b0 = coeff(m0, ov_t, ty); b1 = coeff(m1, ov_t, ty)
            x0 = np.clip(int(np.floor(_cx_at(oh_t, tx, u, b, i, j, k))), 0, W - 1)
            y0 = np.clip(int(np.floor(_cx_at(ov_t, ty, u, b, i, j, k))), 0, H - 1)
            V0, V1, V2 = img[b, 0, x0, y0], img[b, 1, x0, y0], img[b, 2, x0, y0]
            res0 = b0 * (a0 * V0 + a1 * V0) + b1 * (a0 * V1 + a1 * V2)
            res1 = b0 * (a0 * V0 + a1 * V1) + b1 * (a0 * V1 + a1 * V2)
            res2 = b0 * (a0 * V0 + a1 * V1) + b1 * (a0 * V2 + a1 * V2)
            acc += kernels[b, k, i, j] * np.array([res0, res1, res2])
        o = np.float32(acc * 255.0)
        out[b, i, j, :] = o - np.sin(np.float32(2 * np.pi) * o) / np.float32(2 * np.pi)
    return out


# ----------------------------------------------------------------------------
# host-side input packing
# ----------------------------------------------------------------------------

def _pack_inputs(img, kernels, offsets_h, offsets_v):
    B = img.shape[0]
    # A: (B, p, r, m(ae,ao,be,bo), k, jj) fp16
    A = np.empty((B, 128, 2, 4, 9, 128), np.float16)
    ohv = offsets_h.reshape(B, 9, 128, 2, 128, 2)  # (b, plane, p, r, jj, t)
    ovv = offsets_v.reshape(B, 9, 128, 2, 128, 2)
    for k in range(9):
        A[:, :, :, 0, k, :] = ohv[:, SRC0_PLANE[k], :, :, :, SRC0_DELTA[k]]
        A[:, :, :, 1, k, :] = ohv[:, SRC1_PLANE[k], :, :, :, SRC1_DELTA[k]]
        A[:, :, :, 2, k, :] = ovv[:, SRC0_PLANE[k], :, :, :, SRC0_DELTA[k]]
        A[:, :, :, 3, k, :] = ovv[:, SRC1_PLANE[k], :, :, :, SRC1_DELTA[k]]
    A = A.reshape(B, 128, 9216)

    # K: (B, p, half, r, k, jj) fp16
    Kp = kernels.reshape(B, 9, 2, 128, 2, 128).transpose(0, 3, 2, 4, 1, 5)
    Kp = np.ascontiguousarray(Kp.astype(np.float16)).reshape(B, 128, 4608)

    # V tables: (B, t(V0,V1,V2,C01,C12,2V0,2V2), r, k, jj) * 255, fp16,
    # replicated to 32 partitions (device doubles 32->64->128)
    Vt = np.empty((B, 3, 9, 256), np.float32)
    j = np.arange(256)
    for k in range(9):
        Vt[:, :, k, :] = img[:, :, j + TAPS_X[k] + 2, j + TAPS_Y[k] + 2]
    Vt *= 255.0
    # table order: V0 V1 V2 | C12 C01 (C pair is the lo-subtract's in0)
    V5 = np.empty((B, 5, 9, 256), np.float32)
    V5[:, 0:3] = Vt
    V5[:, 3] = Vt[:, 1] + Vt[:, 2]
    V5[:, 4] = Vt[:, 0] + Vt[:, 1]
    # (b, t, k, j) -> (b, t, r, k, jj)
    V5 = V5.reshape(B, 5, 9, 2, 128).transpose(0, 1, 3, 2, 4)
    V5 = V5.astype(np.float16).reshape(B, 1, 11520)
    Vrep = np.ascontiguousarray(np.broadcast_to(V5, (B, 128, 11520)))
    return A, Kp, Vrep


# ----------------------------------------------------------------------------
# entry point
# ----------------------------------------------------------------------------

def kernel(img, kernels, offsets_h, offsets_v):
    img = np.ascontiguousarray(img, np.float32)
    kernels = np.ascontiguousarray(kernels, np.float32)
    offsets_h = np.ascontiguousarray(offsets_h, np.float32)
    offsets_v = np.ascontiguousarray(offsets_v, np.float32)

    A, Kp, Vrep = _pack_inputs(img, kernels, offsets_h, offsets_v)

    nc = _get_nc()
    ident = np.ascontiguousarray(np.eye(128, dtype=np.float16))
    in_maps = [
        {
            "abuf": np.ascontiguousarray(A[b]),
            "kbuf": np.ascontiguousarray(Kp[b]),
            "vbuf": Vrep[b],
            "ibuf": ident,
        }
        for b in range(N_CORES)
    ]
    res = run_bass_kernel_spmd(nc, in_maps, list(range(N_CORES)))
    out = np.stack([res.results[b]["obuf"] for b in range(N_CORES)])  # (8,3,h,w)
    out = np.ascontiguousarray(out.transpose(0, 2, 3, 1))             # (8,h,w,3)
    out = _apply_fixup(out, img, kernels, offsets_h, offsets_v)
    return out.astype(np.float32)
